# revision 27
# baseline (speedup 1.0000x reference)
"""Trainium2 Bass kernel: int4-quantized gate-proj (dequant matmul + qxscale + bias + silu).

Contract: kernel(**inputs) takes FULL unsharded numpy inputs (as produced by
setup_inputs) and returns the FULL [M, N] float32 output.

Sharding: column-parallel (Megatron gate_proj): the N=14336 output dim of
weight_i4 / weight_scale / bias is split into 8 shards of 1792; qx / qxscale
are replicated. Each NeuronCore computes out[:, shard] and the host
concatenates along axis 1.

Layout strategy: the contraction dim K is consumed in a PERMUTED order so
that the packed int4 weights can be dequantized straight into matmul layout
with zero transposes on device:

  k-tile kt = 8*t + j  (t = 128-row block of packed words, j = nibble),
  partition p of tile kt holds original k = 1024*t + 8*p + j.

  - weights: host sends weight_i4.T as wpt[t, p, n]. On device, nibble j of
    wpt[t] is extracted with one DVE tensor_scalar (shift pair) giving the
    int4 for k = 1024t+8p+j at partition p -- already k-on-partitions. The
    group index for partition p is g = 8t + p//16 (constant per tile), so
    the host pre-expands weight_scale into wst[t, p, n] (bf16) and one DVE
    multiply produces each bf16 w-tile. All of wT (14.3 MB) stays resident.
  - activations: host folds qxscale into qx, casts bf16, and pre-arranges
    xt[mb, p, kt, m] with the same k-permutation, blocked by 256-row
    m-blocks so each block is one contiguous 2.1 MB DMA.

Schedule: dequant is DVE-bound (~90 us) while PSUM can only buffer 8 open
accumulation groups, so the dequant is split into 4 column PHASES of 448
and the first 8 m-tiles are computed phase-ordered (kt-outer across the 8
m-tiles, one PSUM bank each) so the PE's in-order queue always has ready
work chasing the dequant stream. The remaining 24 m-tiles run m-major with
all weights resident. Eviction per 448-chunk: DVE bias-add (psum -> sbuf),
ACT native Silu, contiguous-ish store; W-prep DMAs ride the scalar HWDGE
ring while x-block loads ride the sync ring.
"""

import os
import numpy as np
import ml_dtypes

import concourse.bass as bass
import concourse.mybir as mybir
import concourse.tile as tile
from concourse import bacc
from concourse._compat import with_exitstack
from concourse.bass_utils import run_bass_kernel_spmd

M, K, N, G = 4096, 4096, 14336, 128
NCORES = 8
NS = N // NCORES      # 1792 output columns per core
P = 128
KT = K // P           # 32 k-tiles
T = K // 8 // P       # 4 packed-word tiles (each yields 8 k-tiles)
MB = 128              # m-block rows per xt DMA (one m-tile)
NMB = M // MB         # 32
CW = 448              # uniform n-chunk width (PSUM + dequant phases)
NCH = NS // CW        # 4
CWP = 512             # wTc tiles padded to 1024 B so the 896 B moving-
                      # operand reads never straddle a 2 KiB SBUF row
CHASE_MB = 6          # first 6 m-tiles run phase-ordered

f32 = mybir.dt.float32
bf16 = mybir.dt.bfloat16
i32 = mybir.dt.int32

BF16NP = ml_dtypes.bfloat16


@with_exitstack
def _emit(ctx, tc, xt, wpt, wst, bias, out):
    nc = tc.nc
    AL = mybir.AluOpType
    AF = mybir.ActivationFunctionType

    const = ctx.enter_context(tc.tile_pool(name="const", bufs=1))
    wprep = ctx.enter_context(tc.tile_pool(name="wprep", bufs=2))
    wres = ctx.enter_context(tc.tile_pool(name="wres", bufs=1))
    xload = ctx.enter_context(tc.tile_pool(name="xload", bufs=7))
    ev = ctx.enter_context(tc.tile_pool(name="ev", bufs=2))
    psum = ctx.enter_context(tc.tile_pool(name="psum", bufs=8, space="PSUM"))

    # resident dequantized weights, one padded tile per (k-tile, n-chunk)
    wTc = [[wres.tile([P, CWP], bf16, name=f"wT{kt}_{c}", tag=f"wT{kt}_{c}")
            for c in range(NCH)] for kt in range(KT)]
    xq_tiles = {}
    bias_bc = None

    # HAM pre-warm: ~6us of dummy matmuls during the startup DMA window so
    # the PE clock gate is at 8/8 when the first real matmul issues. The
    # dummy PSUM bank fits in the 2 KiB left over by the 8x1792B pool.
    from concourse.masks import make_identity
    junk_l = const.tile([P, P], bf16)
    make_identity(nc, junk_l)
    junk_r = const.tile([P, P], bf16)
    make_identity(nc, junk_r)
    dummy_ps = psum.tile([P, CW], f32, name="ps", tag="ps")
    for _ in range(56):
        nc.tensor.matmul(dummy_ps[:, :P], junk_l[:], junk_r[:],
                         start=True, stop=True)

    def emit_w_chunk_dma(c, t):
        off = c * CW
        wpt_sb = wprep.tile([P, CW], i32, name="wpt_sb")
        nc.sync.dma_start(wpt_sb[:], wpt[t, :, off:off + CW])
        wst_sb = wprep.tile([P, CW], bf16, name="wst_sb")
        nc.sync.dma_start(wst_sb[:], wst[t, :, off:off + CW])
        return wpt_sb, wst_sb

    def emit_dequant_t(c, t, wpt_sb, wst_sb):
        for j in range(8):
            kt = t * 8 + j
            nib = wprep.tile([P, CW], i32, name="nib")
            nc.vector.tensor_scalar(
                out=nib[:], in0=wpt_sb[:],
                scalar1=28 - 4 * j, scalar2=28,
                op0=AL.logical_shift_left, op1=AL.arith_shift_right,
            )
            nc.vector.tensor_tensor(
                out=wTc[kt][c][:, :CW], in0=nib[:], in1=wst_sb[:],
                op=AL.mult,
            )

    def emit_xq_dma(mb):
        xq = xload.tile([P, KT, MB], bf16, name="xq")
        nc.sync.dma_start(xq[:], xt[mb])
        return xq

    def emit_evict(ps, m0, c):
        off = c * CW
        tmp = ev.tile([P, CW], f32, name="tmp", tag="tmp", bufs=3)
        nc.vector.tensor_tensor(out=tmp[:], in0=ps[:],
                                in1=bias_bc[:, off:off + CW], op=AL.add)
        osb = ev.tile([P, CW], f32, name="osb", tag="osb", bufs=3)
        nc.scalar.activation(out=osb[:], in_=tmp[:], func=AF.Silu)
        nc.scalar.dma_start(out[m0:m0 + P, off:off + CW], osb[:])

    # ---- phase 0 dequant, x-chase loads interleaved on the sync ring so
    # the small W-chunk DMAs aren't queued behind megabytes of x ----
    for t in range(T):
        wpt_sb, wst_sb = emit_w_chunk_dma(0, t)
        if t < 3:
            for mb in (2 * t, 2 * t + 1):
                xq_tiles[mb] = emit_xq_dma(mb)
        emit_dequant_t(0, t, wpt_sb, wst_sb)
    # bias broadcast rides the scalar ring (only ACT tables live there now);
    # bf16 to free SBUF for the triple-buffered eviction tiles
    bias_bc = const.tile([P, NS], bf16)
    nc.scalar.dma_start(bias_bc[:], bias.to_broadcast((P, NS)))

    # ---- chase section: 6 m-tiles, phase-ordered, kt-outer ----
    for c in range(NCH):
        ps_ch = [psum.tile([P, CW], f32, name="ps", tag="ps")
                 for _ in range(CHASE_MB)]
        for kt in range(KT):
            for mti in range(CHASE_MB):
                nc.tensor.matmul(
                    ps_ch[mti][:], xq_tiles[mti][:, kt, :],
                    wTc[kt][c][:, :CW],
                    start=(kt == 0), stop=(kt == KT - 1),
                )
        # produce the next phase's chunk before the evicts so the DVE
        # stream stays ahead of the PE, then drain this phase's banks
        if c + 1 < NCH:
            for t in range(T):
                wpt_sb, wst_sb = emit_w_chunk_dma(c + 1, t)
                emit_dequant_t(c + 1, t, wpt_sb, wst_sb)
        for mti in range(CHASE_MB):
            emit_evict(ps_ch[mti], mti * MB, c)

    # ---- steady section: remaining m-blocks, m-major ----
    for mb in range(CHASE_MB, NMB):
        xq = emit_xq_dma(mb)
        m0 = mb * MB
        if mb < NMB - 1:
            pss = [psum.tile([P, CW], f32, name="ps", tag="ps")
                   for _ in range(NCH)]
            for kt in range(KT):
                lhsT = xq[:, kt, :]
                for c in range(NCH):
                    nc.tensor.matmul(
                        pss[c][:], lhsT, wTc[kt][c][:, :CW],
                        start=(kt == 0), stop=(kt == KT - 1),
                    )
            for c in range(NCH):
                emit_evict(pss[c], m0, c)
        else:
            # last m-tile: chunk-major so the evictions of chunks 0-2
            # overlap the remaining chunks' matmuls, trimming the tail
            for c in range(NCH):
                ps = psum.tile([P, CW], f32, name="ps", tag="ps")
                for kt in range(KT):
                    nc.tensor.matmul(
                        ps[:], xq[:, kt, :], wTc[kt][c][:, :CW],
                        start=(kt == 0), stop=(kt == KT - 1),
                    )
                emit_evict(ps, m0, c)


def build_nc():
    nc = bacc.Bacc("TRN2", target_bir_lowering=False, debug=False,
                   enable_asserts=False)
    xt = nc.dram_tensor("xt", [NMB, P, KT, MB], bf16, kind="ExternalInput").ap()
    wpt = nc.dram_tensor("wpt", [T, P, NS], i32, kind="ExternalInput").ap()
    wst = nc.dram_tensor("wst", [T, P, NS], bf16, kind="ExternalInput").ap()
    bias = nc.dram_tensor("bias", [1, NS], bf16, kind="ExternalInput").ap()
    out = nc.dram_tensor("out", [M, NS], f32, kind="ExternalOutput").ap()
    with tile.TileContext(nc) as tc:
        _emit(tc, xt, wpt, wst, bias, out)
    nc.compile()
    return nc


_NC_CACHE = {}


def _get_nc():
    if "nc" not in _NC_CACHE:
        _NC_CACHE["nc"] = build_nc()
    return _NC_CACHE["nc"]


def _prep_x(qx, qxscale):
    """Fold qxscale, cast bf16, and lay out xt[mb, p, kt, m] with the
    permuted k-order (k = 1024t + 8p + j, kt = 8t + j)."""
    xs = (qx * qxscale).astype(BF16NP)          # [M, K]
    # [M, K] -> (mb, mm, t, p, j) -> (mb, p, t, j, mm) -> [NMB, P, KT, MB]
    v = xs.reshape(NMB, MB, T, P, 8)
    return np.ascontiguousarray(v.transpose(0, 3, 2, 4, 1)).reshape(
        NMB, P, KT, MB)


def _make_in_maps(qx, qxscale, weight_i4, weight_scale, bias):
    xt = _prep_x(qx, qxscale)
    in_maps = []
    for c in range(NCORES):
        sl = slice(c * NS, (c + 1) * NS)
        wpt = np.ascontiguousarray(weight_i4[sl].T).reshape(T, P, NS)
        wst = np.ascontiguousarray(
            np.repeat(weight_scale[sl].T, 16, axis=0)).astype(
            BF16NP).reshape(T, P, NS)
        in_maps.append({
            "xt": xt,
            "wpt": wpt,
            "wst": wst,
            "bias": np.ascontiguousarray(bias[sl]).astype(
                BF16NP).reshape(1, NS),
        })
    return in_maps


def run(qx, qxscale, weight_i4, weight_scale, bias, trace=False, **spmd_kwargs):
    nc = _get_nc()
    in_maps = _make_in_maps(qx, qxscale, weight_i4, weight_scale, bias)
    res = run_bass_kernel_spmd(nc, in_maps, core_ids=list(range(NCORES)),
                               trace=trace, **spmd_kwargs)
    out = np.concatenate([res.results[c]["out"] for c in range(NCORES)],
                         axis=1)
    return out, res


def kernel(qx, qxscale, weight_i4, weight_scale, bias, group_size=G):
    gs = int(np.asarray(group_size))
    assert gs == G, f"kernel hardcodes group_size={G}, got {gs}"
    qx = np.ascontiguousarray(np.asarray(qx, dtype=np.float32))
    qxscale = np.ascontiguousarray(
        np.asarray(qxscale, dtype=np.float32).reshape(M, 1))
    weight_i4 = np.ascontiguousarray(np.asarray(weight_i4, dtype=np.int32))
    weight_scale = np.ascontiguousarray(
        np.asarray(weight_scale, dtype=np.float32))
    bias = np.ascontiguousarray(
        np.asarray(bias, dtype=np.float32).reshape(-1))
    out, _ = run(qx, qxscale, weight_i4, weight_scale, bias,
                 trace=bool(int(os.environ.get("GATEPROJ_TRACE", "0"))))
    return out


# revision 29
# speedup vs baseline: 1.0009x; 1.0009x over previous
"""Trainium2 Bass kernel: int4-quantized gate-proj (dequant matmul + qxscale + bias + silu).

Contract: kernel(**inputs) takes FULL unsharded numpy inputs (as produced by
setup_inputs) and returns the FULL [M, N] float32 output.

Sharding: column-parallel (Megatron gate_proj): the N=14336 output dim of
weight_i4 / weight_scale / bias is split into 8 shards of 1792; qx / qxscale
are replicated. Each NeuronCore computes out[:, shard] and the host
concatenates along axis 1.

Layout strategy: the contraction dim K is consumed in a PERMUTED order so
that the packed int4 weights can be dequantized straight into matmul layout
with zero transposes on device:

  k-tile kt = 8*t + j  (t = 128-row block of packed words, j = nibble),
  partition p of tile kt holds original k = 1024*t + 8*p + j.

  - weights: host sends weight_i4.T as wpt[t, p, n]. On device, nibble j of
    wpt[t] is extracted with one DVE tensor_scalar (shift pair) giving the
    int4 for k = 1024t+8p+j at partition p -- already k-on-partitions. The
    group index for partition p is g = 8t + p//16 (constant per tile), so
    the host pre-expands weight_scale into wst[t, p, n] (bf16) and one DVE
    multiply produces each bf16 w-tile. All of wT (14.3 MB) stays resident.
  - activations: host folds qxscale into qx, casts bf16, and pre-arranges
    xt[mb, p, kt, m] with the same k-permutation, blocked by 256-row
    m-blocks so each block is one contiguous 2.1 MB DMA.

Schedule: dequant is DVE-bound (~90 us) while PSUM can only buffer 8 open
accumulation groups, so the dequant is split into 4 column PHASES of 448
and the first 8 m-tiles are computed phase-ordered (kt-outer across the 8
m-tiles, one PSUM bank each) so the PE's in-order queue always has ready
work chasing the dequant stream. The remaining 24 m-tiles run m-major with
all weights resident. Eviction per 448-chunk: DVE bias-add (psum -> sbuf),
ACT native Silu, contiguous-ish store; W-prep DMAs ride the scalar HWDGE
ring while x-block loads ride the sync ring.
"""

import os
import numpy as np
import ml_dtypes

import concourse.bass as bass
import concourse.mybir as mybir
import concourse.tile as tile
from concourse import bacc
from concourse._compat import with_exitstack
from concourse.bass_utils import run_bass_kernel_spmd

M, K, N, G = 4096, 4096, 14336, 128
NCORES = 8
NS = N // NCORES      # 1792 output columns per core
P = 128
KT = K // P           # 32 k-tiles
T = K // 8 // P       # 4 packed-word tiles (each yields 8 k-tiles)
MB = 128              # m-block rows per xt DMA (one m-tile)
NMB = M // MB         # 32
CW = 448              # uniform n-chunk width (PSUM + dequant phases)
NCH = NS // CW        # 4
CWP = 512             # wTc tiles padded to 1024 B so the 896 B moving-
                      # operand reads never straddle a 2 KiB SBUF row
CHASE_MB = 6          # first 6 m-tiles run phase-ordered

f32 = mybir.dt.float32
bf16 = mybir.dt.bfloat16
i32 = mybir.dt.int32

BF16NP = ml_dtypes.bfloat16


@with_exitstack
def _emit(ctx, tc, xt, wpt, wst, bias, out):
    nc = tc.nc
    AL = mybir.AluOpType
    AF = mybir.ActivationFunctionType

    const = ctx.enter_context(tc.tile_pool(name="const", bufs=1))
    wprep = ctx.enter_context(tc.tile_pool(name="wprep", bufs=2))
    wres = ctx.enter_context(tc.tile_pool(name="wres", bufs=1))
    xload = ctx.enter_context(tc.tile_pool(name="xload", bufs=7))
    ev = ctx.enter_context(tc.tile_pool(name="ev", bufs=2))
    psum = ctx.enter_context(tc.tile_pool(name="psum", bufs=8, space="PSUM"))

    # resident dequantized weights, one padded tile per (k-tile, n-chunk)
    wTc = [[wres.tile([P, CWP], bf16, name=f"wT{kt}_{c}", tag=f"wT{kt}_{c}")
            for c in range(NCH)] for kt in range(KT)]
    xq_tiles = {}
    bias_bc = None

    # HAM pre-warm: ~6us of dummy matmuls during the startup DMA window so
    # the PE clock gate is at 8/8 when the first real matmul issues. The
    # dummy PSUM bank fits in the 2 KiB left over by the 8x1792B pool.
    from concourse.masks import make_identity
    junk_l = const.tile([P, P], bf16)
    make_identity(nc, junk_l)
    junk_r = const.tile([P, P], bf16)
    make_identity(nc, junk_r)
    dummy_ps = psum.tile([P, CW], f32, name="ps", tag="ps")
    for _ in range(56):
        nc.tensor.matmul(dummy_ps[:, :P], junk_l[:], junk_r[:],
                         start=True, stop=True)

    def emit_w_chunk_dma(c, t):
        # wst first: the dequant multiply waits on BOTH receipts, and the
        # shift-extract in between hides only the wpt one
        off = c * CW
        wst_sb = wprep.tile([P, CW], bf16, name="wst_sb")
        nc.sync.dma_start(wst_sb[:], wst[t, :, off:off + CW])
        wpt_sb = wprep.tile([P, CW], i32, name="wpt_sb")
        nc.sync.dma_start(wpt_sb[:], wpt[t, :, off:off + CW])
        return wpt_sb, wst_sb

    def emit_dequant_t(c, t, wpt_sb, wst_sb):
        for j in range(8):
            kt = t * 8 + j
            nib = wprep.tile([P, CW], i32, name="nib")
            nc.vector.tensor_scalar(
                out=nib[:], in0=wpt_sb[:],
                scalar1=28 - 4 * j, scalar2=28,
                op0=AL.logical_shift_left, op1=AL.arith_shift_right,
            )
            nc.vector.tensor_tensor(
                out=wTc[kt][c][:, :CW], in0=nib[:], in1=wst_sb[:],
                op=AL.mult,
            )

    def emit_xq_dma(mb):
        xq = xload.tile([P, KT, MB], bf16, name="xq")
        nc.sync.dma_start(xq[:], xt[mb])
        return xq

    def emit_evict(ps, m0, c):
        off = c * CW
        tmp = ev.tile([P, CW], f32, name="tmp", tag="tmp", bufs=3)
        nc.vector.tensor_tensor(out=tmp[:], in0=ps[:],
                                in1=bias_bc[:, off:off + CW], op=AL.add)
        osb = ev.tile([P, CW], f32, name="osb", tag="osb", bufs=3)
        nc.scalar.activation(out=osb[:], in_=tmp[:], func=AF.Silu)
        nc.scalar.dma_start(out[m0:m0 + P, off:off + CW], osb[:])

    # ---- phase 0 dequant, x-chase loads interleaved on the sync ring so
    # the small W-chunk DMAs aren't queued behind megabytes of x ----
    for t in range(T):
        wpt_sb, wst_sb = emit_w_chunk_dma(0, t)
        if t < 3:
            for mb in (2 * t, 2 * t + 1):
                xq_tiles[mb] = emit_xq_dma(mb)
        emit_dequant_t(0, t, wpt_sb, wst_sb)
    # bias broadcast rides the scalar ring (only ACT tables live there now);
    # bf16 to free SBUF for the triple-buffered eviction tiles
    bias_bc = const.tile([P, NS], bf16)
    nc.scalar.dma_start(bias_bc[:], bias.to_broadcast((P, NS)))

    # ---- chase section: 6 m-tiles, phase-ordered, kt-outer ----
    for c in range(NCH):
        ps_ch = [psum.tile([P, CW], f32, name="ps", tag="ps")
                 for _ in range(CHASE_MB)]
        for kt in range(KT):
            for mti in range(CHASE_MB):
                nc.tensor.matmul(
                    ps_ch[mti][:], xq_tiles[mti][:, kt, :],
                    wTc[kt][c][:, :CW],
                    start=(kt == 0), stop=(kt == KT - 1),
                )
        # produce the next phase's chunk before the evicts so the DVE
        # stream stays ahead of the PE, then drain this phase's banks
        if c + 1 < NCH:
            for t in range(T):
                wpt_sb, wst_sb = emit_w_chunk_dma(c + 1, t)
                emit_dequant_t(c + 1, t, wpt_sb, wst_sb)
        for mti in range(CHASE_MB):
            emit_evict(ps_ch[mti], mti * MB, c)

    # ---- steady section: remaining m-blocks, m-major ----
    for mb in range(CHASE_MB, NMB):
        xq = emit_xq_dma(mb)
        m0 = mb * MB
        if mb < NMB - 1:
            pss = [psum.tile([P, CW], f32, name="ps", tag="ps")
                   for _ in range(NCH)]
            for kt in range(KT):
                lhsT = xq[:, kt, :]
                for c in range(NCH):
                    nc.tensor.matmul(
                        pss[c][:], lhsT, wTc[kt][c][:, :CW],
                        start=(kt == 0), stop=(kt == KT - 1),
                    )
            for c in range(NCH):
                emit_evict(pss[c], m0, c)
        else:
            # last m-tile: chunk-major so the evictions of chunks 0-2
            # overlap the remaining chunks' matmuls; the final chunk is
            # split in half so its eviction pipelines too
            for c in range(NCH - 1):
                ps = psum.tile([P, CW], f32, name="ps", tag="ps")
                for kt in range(KT):
                    nc.tensor.matmul(
                        ps[:], xq[:, kt, :], wTc[kt][c][:, :CW],
                        start=(kt == 0), stop=(kt == KT - 1),
                    )
                emit_evict(ps, m0, c)
            c = NCH - 1
            HW2 = CW // 2
            for half in range(2):
                ps = psum.tile([P, CW], f32, name="ps", tag="ps")
                w0 = half * HW2
                for kt in range(KT):
                    nc.tensor.matmul(
                        ps[:, :HW2], xq[:, kt, :],
                        wTc[kt][c][:, w0:w0 + HW2],
                        start=(kt == 0), stop=(kt == KT - 1),
                    )
                off = c * CW + w0
                tmp = ev.tile([P, CW], f32, name="tmp", tag="tmp", bufs=3)
                nc.vector.tensor_tensor(out=tmp[:, :HW2], in0=ps[:, :HW2],
                                        in1=bias_bc[:, off:off + HW2],
                                        op=AL.add)
                osb = ev.tile([P, CW], f32, name="osb", tag="osb", bufs=3)
                nc.scalar.activation(out=osb[:, :HW2], in_=tmp[:, :HW2],
                                     func=AF.Silu)
                nc.scalar.dma_start(out[m0:m0 + P, off:off + HW2],
                                    osb[:, :HW2])


def build_nc():
    nc = bacc.Bacc("TRN2", target_bir_lowering=False, debug=False,
                   enable_asserts=False)
    xt = nc.dram_tensor("xt", [NMB, P, KT, MB], bf16, kind="ExternalInput").ap()
    wpt = nc.dram_tensor("wpt", [T, P, NS], i32, kind="ExternalInput").ap()
    wst = nc.dram_tensor("wst", [T, P, NS], bf16, kind="ExternalInput").ap()
    bias = nc.dram_tensor("bias", [1, NS], bf16, kind="ExternalInput").ap()
    out = nc.dram_tensor("out", [M, NS], f32, kind="ExternalOutput").ap()
    with tile.TileContext(nc) as tc:
        _emit(tc, xt, wpt, wst, bias, out)
    nc.compile()
    return nc


_NC_CACHE = {}


def _get_nc():
    if "nc" not in _NC_CACHE:
        _NC_CACHE["nc"] = build_nc()
    return _NC_CACHE["nc"]


def _prep_x(qx, qxscale):
    """Fold qxscale, cast bf16, and lay out xt[mb, p, kt, m] with the
    permuted k-order (k = 1024t + 8p + j, kt = 8t + j)."""
    xs = (qx * qxscale).astype(BF16NP)          # [M, K]
    # [M, K] -> (mb, mm, t, p, j) -> (mb, p, t, j, mm) -> [NMB, P, KT, MB]
    v = xs.reshape(NMB, MB, T, P, 8)
    return np.ascontiguousarray(v.transpose(0, 3, 2, 4, 1)).reshape(
        NMB, P, KT, MB)


def _make_in_maps(qx, qxscale, weight_i4, weight_scale, bias):
    xt = _prep_x(qx, qxscale)
    in_maps = []
    for c in range(NCORES):
        sl = slice(c * NS, (c + 1) * NS)
        wpt = np.ascontiguousarray(weight_i4[sl].T).reshape(T, P, NS)
        wst = np.ascontiguousarray(
            np.repeat(weight_scale[sl].T, 16, axis=0)).astype(
            BF16NP).reshape(T, P, NS)
        in_maps.append({
            "xt": xt,
            "wpt": wpt,
            "wst": wst,
            "bias": np.ascontiguousarray(bias[sl]).astype(
                BF16NP).reshape(1, NS),
        })
    return in_maps


def run(qx, qxscale, weight_i4, weight_scale, bias, trace=False, **spmd_kwargs):
    nc = _get_nc()
    in_maps = _make_in_maps(qx, qxscale, weight_i4, weight_scale, bias)
    res = run_bass_kernel_spmd(nc, in_maps, core_ids=list(range(NCORES)),
                               trace=trace, **spmd_kwargs)
    out = np.concatenate([res.results[c]["out"] for c in range(NCORES)],
                         axis=1)
    return out, res


def kernel(qx, qxscale, weight_i4, weight_scale, bias, group_size=G):
    gs = int(np.asarray(group_size))
    assert gs == G, f"kernel hardcodes group_size={G}, got {gs}"
    qx = np.ascontiguousarray(np.asarray(qx, dtype=np.float32))
    qxscale = np.ascontiguousarray(
        np.asarray(qxscale, dtype=np.float32).reshape(M, 1))
    weight_i4 = np.ascontiguousarray(np.asarray(weight_i4, dtype=np.int32))
    weight_scale = np.ascontiguousarray(
        np.asarray(weight_scale, dtype=np.float32))
    bias = np.ascontiguousarray(
        np.asarray(bias, dtype=np.float32).reshape(-1))
    out, _ = run(qx, qxscale, weight_i4, weight_scale, bias,
                 trace=bool(int(os.environ.get("GATEPROJ_TRACE", "0"))))
    return out


# revision 30
# speedup vs baseline: 1.0011x; 1.0002x over previous
"""Trainium2 Bass kernel: int4-quantized gate-proj (dequant matmul + qxscale + bias + silu).

Contract: kernel(**inputs) takes FULL unsharded numpy inputs (as produced by
setup_inputs) and returns the FULL [M, N] float32 output.

Sharding: column-parallel (Megatron gate_proj): the N=14336 output dim of
weight_i4 / weight_scale / bias is split into 8 shards of 1792; qx / qxscale
are replicated. Each NeuronCore computes out[:, shard] and the host
concatenates along axis 1.

Layout strategy: the contraction dim K is consumed in a PERMUTED order so
that the packed int4 weights can be dequantized straight into matmul layout
with zero transposes on device:

  k-tile kt = 8*t + j  (t = 128-row block of packed words, j = nibble),
  partition p of tile kt holds original k = 1024*t + 8*p + j.

  - weights: host sends weight_i4.T as wpt[t, p, n]. On device, nibble j of
    wpt[t] is extracted with one DVE tensor_scalar (shift pair) giving the
    int4 for k = 1024t+8p+j at partition p -- already k-on-partitions. The
    group index for partition p is g = 8t + p//16 (constant per tile), so
    the host pre-expands weight_scale into wst[t, p, n] (bf16) and one DVE
    multiply produces each bf16 w-tile. All of wT (14.3 MB) stays resident.
  - activations: host folds qxscale into qx, casts bf16, and pre-arranges
    xt[mb, p, kt, m] with the same k-permutation, blocked by 256-row
    m-blocks so each block is one contiguous 2.1 MB DMA.

Schedule: dequant is DVE-bound (~90 us) while PSUM can only buffer 8 open
accumulation groups, so the dequant is split into 4 column PHASES of 448
and the first 8 m-tiles are computed phase-ordered (kt-outer across the 8
m-tiles, one PSUM bank each) so the PE's in-order queue always has ready
work chasing the dequant stream. The remaining 24 m-tiles run m-major with
all weights resident. Eviction per 448-chunk: DVE bias-add (psum -> sbuf),
ACT native Silu, contiguous-ish store; W-prep DMAs ride the scalar HWDGE
ring while x-block loads ride the sync ring.
"""

import os
import numpy as np
import ml_dtypes

import concourse.bass as bass
import concourse.mybir as mybir
import concourse.tile as tile
from concourse import bacc
from concourse._compat import with_exitstack
from concourse.bass_utils import run_bass_kernel_spmd

M, K, N, G = 4096, 4096, 14336, 128
NCORES = 8
NS = N // NCORES      # 1792 output columns per core
P = 128
KT = K // P           # 32 k-tiles
T = K // 8 // P       # 4 packed-word tiles (each yields 8 k-tiles)
MB = 128              # m-block rows per xt DMA (one m-tile)
NMB = M // MB         # 32
CW = 448              # uniform n-chunk width (PSUM + dequant phases)
NCH = NS // CW        # 4
CWP = 512             # wTc tiles padded to 1024 B so the 896 B moving-
                      # operand reads never straddle a 2 KiB SBUF row
CHASE_MB = 6          # first 6 m-tiles run phase-ordered

f32 = mybir.dt.float32
bf16 = mybir.dt.bfloat16
i32 = mybir.dt.int32

BF16NP = ml_dtypes.bfloat16


@with_exitstack
def _emit(ctx, tc, xt, wpt, wst, bias, out):
    nc = tc.nc
    AL = mybir.AluOpType
    AF = mybir.ActivationFunctionType

    const = ctx.enter_context(tc.tile_pool(name="const", bufs=1))
    wprep = ctx.enter_context(tc.tile_pool(name="wprep", bufs=2))
    wres = ctx.enter_context(tc.tile_pool(name="wres", bufs=1))
    xload = ctx.enter_context(tc.tile_pool(name="xload", bufs=7))
    ev = ctx.enter_context(tc.tile_pool(name="ev", bufs=2))
    psum = ctx.enter_context(tc.tile_pool(name="psum", bufs=8, space="PSUM"))

    # resident dequantized weights, one padded tile per (k-tile, n-chunk)
    wTc = [[wres.tile([P, CWP], bf16, name=f"wT{kt}_{c}", tag=f"wT{kt}_{c}")
            for c in range(NCH)] for kt in range(KT)]
    xq_tiles = {}
    bias_bc = None

    # HAM pre-warm: ~6us of dummy matmuls during the startup DMA window so
    # the PE clock gate is at 8/8 when the first real matmul issues. The
    # dummy PSUM bank fits in the 2 KiB left over by the 8x1792B pool.
    from concourse.masks import make_identity
    junk_l = const.tile([P, P], bf16)
    make_identity(nc, junk_l)
    junk_r = const.tile([P, P], bf16)
    make_identity(nc, junk_r)
    dummy_ps = psum.tile([P, CW], f32, name="ps", tag="ps")
    for _ in range(56):
        nc.tensor.matmul(dummy_ps[:, :P], junk_l[:], junk_r[:],
                         start=True, stop=True)

    def emit_w_chunk_dma(c, t):
        # wst first: the dequant multiply waits on BOTH receipts, and the
        # shift-extract in between hides only the wpt one
        off = c * CW
        wst_sb = wprep.tile([P, CW], bf16, name="wst_sb")
        nc.sync.dma_start(wst_sb[:], wst[t, :, off:off + CW])
        wpt_sb = wprep.tile([P, CW], i32, name="wpt_sb")
        nc.sync.dma_start(wpt_sb[:], wpt[t, :, off:off + CW])
        return wpt_sb, wst_sb

    def emit_dequant_t(c, t, wpt_sb, wst_sb):
        for j in range(8):
            kt = t * 8 + j
            nib = wprep.tile([P, CW], i32, name="nib")
            nc.vector.tensor_scalar(
                out=nib[:], in0=wpt_sb[:],
                scalar1=28 - 4 * j, scalar2=28,
                op0=AL.logical_shift_left, op1=AL.arith_shift_right,
            )
            nc.vector.tensor_tensor(
                out=wTc[kt][c][:, :CW], in0=nib[:], in1=wst_sb[:],
                op=AL.mult,
            )

    def emit_xq_dma(mb):
        # two kt-half transfers: the first half's matmuls can start while
        # the second half is still in flight (contiguous subtile ranges)
        xq = xload.tile([P, KT, MB], bf16, name="xq")
        nc.sync.dma_start(xq[:, :KT // 2, :], xt[mb, :, :KT // 2, :])
        nc.sync.dma_start(xq[:, KT // 2:, :], xt[mb, :, KT // 2:, :])
        return xq

    def emit_evict(ps, m0, c):
        off = c * CW
        tmp = ev.tile([P, CW], f32, name="tmp", tag="tmp", bufs=3)
        nc.vector.tensor_tensor(out=tmp[:], in0=ps[:],
                                in1=bias_bc[:, off:off + CW], op=AL.add)
        osb = ev.tile([P, CW], f32, name="osb", tag="osb", bufs=3)
        nc.scalar.activation(out=osb[:], in_=tmp[:], func=AF.Silu)
        nc.scalar.dma_start(out[m0:m0 + P, off:off + CW], osb[:])

    # ---- phase 0 dequant, x-chase loads interleaved on the sync ring so
    # the small W-chunk DMAs aren't queued behind megabytes of x ----
    for t in range(T):
        wpt_sb, wst_sb = emit_w_chunk_dma(0, t)
        if t < 3:
            for mb in (2 * t, 2 * t + 1):
                xq_tiles[mb] = emit_xq_dma(mb)
        emit_dequant_t(0, t, wpt_sb, wst_sb)
    # bias broadcast rides the scalar ring (only ACT tables live there now);
    # bf16 to free SBUF for the triple-buffered eviction tiles
    bias_bc = const.tile([P, NS], bf16)
    nc.scalar.dma_start(bias_bc[:], bias.to_broadcast((P, NS)))

    # ---- chase section: 6 m-tiles, phase-ordered, kt-outer ----
    for c in range(NCH):
        ps_ch = [psum.tile([P, CW], f32, name="ps", tag="ps")
                 for _ in range(CHASE_MB)]
        for kt in range(KT):
            for mti in range(CHASE_MB):
                nc.tensor.matmul(
                    ps_ch[mti][:], xq_tiles[mti][:, kt, :],
                    wTc[kt][c][:, :CW],
                    start=(kt == 0), stop=(kt == KT - 1),
                )
        # produce the next phase's chunk before the evicts so the DVE
        # stream stays ahead of the PE, then drain this phase's banks
        if c + 1 < NCH:
            for t in range(T):
                wpt_sb, wst_sb = emit_w_chunk_dma(c + 1, t)
                emit_dequant_t(c + 1, t, wpt_sb, wst_sb)
        for mti in range(CHASE_MB):
            emit_evict(ps_ch[mti], mti * MB, c)

    # ---- steady section: remaining m-blocks, m-major ----
    for mb in range(CHASE_MB, NMB):
        xq = emit_xq_dma(mb)
        m0 = mb * MB
        if mb < NMB - 1:
            pss = [psum.tile([P, CW], f32, name="ps", tag="ps")
                   for _ in range(NCH)]
            for kt in range(KT):
                lhsT = xq[:, kt, :]
                for c in range(NCH):
                    nc.tensor.matmul(
                        pss[c][:], lhsT, wTc[kt][c][:, :CW],
                        start=(kt == 0), stop=(kt == KT - 1),
                    )
            for c in range(NCH):
                emit_evict(pss[c], m0, c)
        else:
            # last m-tile: chunk-major so the evictions of chunks 0-2
            # overlap the remaining chunks' matmuls; the final chunk is
            # split in half so its eviction pipelines too
            for c in range(NCH - 1):
                ps = psum.tile([P, CW], f32, name="ps", tag="ps")
                for kt in range(KT):
                    nc.tensor.matmul(
                        ps[:], xq[:, kt, :], wTc[kt][c][:, :CW],
                        start=(kt == 0), stop=(kt == KT - 1),
                    )
                emit_evict(ps, m0, c)
            c = NCH - 1
            HW2 = CW // 2
            for half in range(2):
                ps = psum.tile([P, CW], f32, name="ps", tag="ps")
                w0 = half * HW2
                for kt in range(KT):
                    nc.tensor.matmul(
                        ps[:, :HW2], xq[:, kt, :],
                        wTc[kt][c][:, w0:w0 + HW2],
                        start=(kt == 0), stop=(kt == KT - 1),
                    )
                off = c * CW + w0
                tmp = ev.tile([P, CW], f32, name="tmp", tag="tmp", bufs=3)
                nc.vector.tensor_tensor(out=tmp[:, :HW2], in0=ps[:, :HW2],
                                        in1=bias_bc[:, off:off + HW2],
                                        op=AL.add)
                osb = ev.tile([P, CW], f32, name="osb", tag="osb", bufs=3)
                nc.scalar.activation(out=osb[:, :HW2], in_=tmp[:, :HW2],
                                     func=AF.Silu)
                nc.scalar.dma_start(out[m0:m0 + P, off:off + HW2],
                                    osb[:, :HW2])


def build_nc():
    nc = bacc.Bacc("TRN2", target_bir_lowering=False, debug=False,
                   enable_asserts=False)
    xt = nc.dram_tensor("xt", [NMB, P, KT, MB], bf16, kind="ExternalInput").ap()
    wpt = nc.dram_tensor("wpt", [T, P, NS], i32, kind="ExternalInput").ap()
    wst = nc.dram_tensor("wst", [T, P, NS], bf16, kind="ExternalInput").ap()
    bias = nc.dram_tensor("bias", [1, NS], bf16, kind="ExternalInput").ap()
    out = nc.dram_tensor("out", [M, NS], f32, kind="ExternalOutput").ap()
    with tile.TileContext(nc) as tc:
        _emit(tc, xt, wpt, wst, bias, out)
    nc.compile()
    return nc


_NC_CACHE = {}


def _get_nc():
    if "nc" not in _NC_CACHE:
        _NC_CACHE["nc"] = build_nc()
    return _NC_CACHE["nc"]


def _prep_x(qx, qxscale):
    """Fold qxscale, cast bf16, and lay out xt[mb, p, kt, m] with the
    permuted k-order (k = 1024t + 8p + j, kt = 8t + j)."""
    xs = (qx * qxscale).astype(BF16NP)          # [M, K]
    # [M, K] -> (mb, mm, t, p, j) -> (mb, p, t, j, mm) -> [NMB, P, KT, MB]
    v = xs.reshape(NMB, MB, T, P, 8)
    return np.ascontiguousarray(v.transpose(0, 3, 2, 4, 1)).reshape(
        NMB, P, KT, MB)


def _make_in_maps(qx, qxscale, weight_i4, weight_scale, bias):
    xt = _prep_x(qx, qxscale)
    in_maps = []
    for c in range(NCORES):
        sl = slice(c * NS, (c + 1) * NS)
        wpt = np.ascontiguousarray(weight_i4[sl].T).reshape(T, P, NS)
        wst = np.ascontiguousarray(
            np.repeat(weight_scale[sl].T, 16, axis=0)).astype(
            BF16NP).reshape(T, P, NS)
        in_maps.append({
            "xt": xt,
            "wpt": wpt,
            "wst": wst,
            "bias": np.ascontiguousarray(bias[sl]).astype(
                BF16NP).reshape(1, NS),
        })
    return in_maps


def run(qx, qxscale, weight_i4, weight_scale, bias, trace=False, **spmd_kwargs):
    nc = _get_nc()
    in_maps = _make_in_maps(qx, qxscale, weight_i4, weight_scale, bias)
    res = run_bass_kernel_spmd(nc, in_maps, core_ids=list(range(NCORES)),
                               trace=trace, **spmd_kwargs)
    out = np.concatenate([res.results[c]["out"] for c in range(NCORES)],
                         axis=1)
    return out, res


def kernel(qx, qxscale, weight_i4, weight_scale, bias, group_size=G):
    gs = int(np.asarray(group_size))
    assert gs == G, f"kernel hardcodes group_size={G}, got {gs}"
    qx = np.ascontiguousarray(np.asarray(qx, dtype=np.float32))
    qxscale = np.ascontiguousarray(
        np.asarray(qxscale, dtype=np.float32).reshape(M, 1))
    weight_i4 = np.ascontiguousarray(np.asarray(weight_i4, dtype=np.int32))
    weight_scale = np.ascontiguousarray(
        np.asarray(weight_scale, dtype=np.float32))
    bias = np.ascontiguousarray(
        np.asarray(bias, dtype=np.float32).reshape(-1))
    out, _ = run(qx, qxscale, weight_i4, weight_scale, bias,
                 trace=bool(int(os.environ.get("GATEPROJ_TRACE", "0"))))
    return out
